# revision 21
# baseline (speedup 1.0000x reference)
"""DeepSeekV3 MLA attention prefill kernel for 8 Trainium2 NeuronCores.

Sharding: sequence-parallel low-rank input projections (q_a / kv_a),
AllGather of the shared latents (kv first, hidden under q_a compute; q AG
hidden under kv decompression), tensor-parallel over heads (4 heads/core)
for q_b / kv_b decompression and attention, per-head AllToAll to
redistribute attention outputs seq-wise (8x less wire than AllGather), and
a 4x2 (seq-block x H-half) sharded o_proj with streamed weights.

Matmuls: projections bf16, score-nope fp32r, score-pe bf16, AV bf16.
"""

import sys

sys.path.insert(0, "/opt/trn_rl_repo")

import numpy as np
import ml_dtypes

import concourse.bass as bass  # noqa: F401
import concourse.mybir as mybir
from concourse import bacc
import functools as _ft

# Route every Exp to natural_log_exp_and_others (which genuinely contains
# Exp) so the attention's Ln/Exp mix stays in ONE ACT table set - otherwise
# the table-load pass alternates sets per softmax tail (~2.7us each).
_orig_get_tables = bacc.get_activation_tables


@_ft.cache
def _exp_unified_tables(module_arch):
    AFT = mybir.ActivationFunctionType
    out = {}
    for name, fns in _orig_get_tables(module_arch).items():
        fns = set(fns)
        if name != "natural_log_exp_and_others":
            fns.discard(AFT.Exp)
        out[name] = fns
    return out


bacc.get_activation_tables = _exp_unified_tables
from concourse.bass import ds, ts
from concourse.tile import TileContext
from concourse.bass_utils import run_bass_kernel_spmd
from contextlib import ExitStack

F = mybir.dt.float32
BF = mybir.dt.bfloat16
R = mybir.dt.float32r
I32 = mybir.dt.int32
AF = mybir.ActivationFunctionType
ALU = mybir.AluOpType

NCORES = 8
B, S, H = 1, 2048, 4096
N_HEADS = 32
HPC = N_HEADS // NCORES          # heads per core = 4
SL = S // NCORES                 # sequence rows per core = 256
QR, KR = 1536, 512
DR, DN, DV = 64, 128, 128
QD = DN + DR                     # 192
SCALE = QD ** -0.5
EPS = 1e-6
THETA = 10000.0
TWO_PI = float(2.0 * np.pi)
QBLK = 512                       # o_proj q-block per core (4-way over seq)
HHALF = H // 2                   # o_proj H columns per core (2-way)

LAST_RESULT = None               # test harness reads exec_time_ns from here
_CACHED_NC = None
_UID = [0]


def _uid():
    _UID[0] += 1
    return _UID[0]


def _emit_range_reduce(nc, pool, t_ap, width):
    """In-place wrap t_ap (f32, [128, width]) to [-pi, pi]. f32->i32 copy
    rounds to nearest (verified on HW)."""
    tn = pool.tile([128, width], F, tag=f"rr_f_{width}", name=f"rrf{_uid()}")
    ti = pool.tile([128, width], I32, tag=f"rr_i_{width}", name=f"rri{_uid()}")
    nc.vector.tensor_scalar_mul(tn[:], t_ap, 1.0 / TWO_PI)
    nc.vector.tensor_copy(ti[:], tn[:])
    nc.vector.tensor_copy(tn[:], ti[:])
    nc.vector.tensor_scalar_mul(tn[:], tn[:], -TWO_PI)
    nc.vector.tensor_tensor(t_ap, t_ap, tn[:], ALU.add)


def _build_program():
    nc = bacc.Bacc(None, target_bir_lowering=False, num_devices=NCORES)

    # ---------------- DRAM declarations ----------------
    xT = nc.dram_tensor("xT", [128, H // 128, SL], BF, kind="ExternalInput")
    qawT = nc.dram_tensor("qawT", [3, 128, H // 128, 512], BF, kind="ExternalInput")
    kvawT = nc.dram_tensor("kvawT", [128, H // 128, KR + DR], BF, kind="ExternalInput")
    qbwT = nc.dram_tensor("qbwT", [QR, 768], BF, kind="ExternalInput")
    kvbwT = nc.dram_tensor("kvbwT", [KR, 1024], BF, kind="ExternalInput")
    owT = nc.dram_tensor("owT", [16, 4096, 128], BF, kind="ExternalInput")
    pos_all = nc.dram_tensor("pos_all", [1, S], I32, kind="ExternalInput")
    pos_loc = nc.dram_tensor("pos_loc", [SL], I32, kind="ExternalInput")
    ident_d = nc.dram_tensor("ident", [128, 128], F, kind="ExternalInput")
    triu_d = nc.dram_tensor("triu", [128, 128], BF, kind="ExternalInput")
    if32_d = nc.dram_tensor("if32", [128, 32], F, kind="ExternalInput")
    if128_d = nc.dram_tensor("if128", [128, 1], F, kind="ExternalInput")
    out_d = nc.dram_tensor("out", [HHALF, QBLK], F, kind="ExternalOutput")

    g1kv_src = nc.dram_tensor("g1kv_src", [KR + DR, SL], BF)
    g1kv = nc.dram_tensor("g1kv", [NCORES, KR + DR, SL], BF, addr_space="Shared")
    g1q_src = nc.dram_tensor("g1q_src", [QR, SL], BF)
    g1q = nc.dram_tensor("g1q", [NCORES, QR, SL], BF, addr_space="Shared")
    # per-head AllToAll buffers: shard j holds this core's head-h attention
    # output for q-range (j % 4); shards j and j+4 are identical copies so
    # cores j and j+4 (the two H-halves) both receive that q-range.
    a2a_src = [nc.dram_tensor(f"a2as{h}", [NCORES, DV, QBLK], BF) for h in range(HPC)]
    a2a_out = [
        nc.dram_tensor(f"a2ao{h}", [NCORES, DV, QBLK], BF) for h in range(HPC)
    ]
    RG = [list(range(NCORES))]
    NKT = H // 128  # 32 k-tiles over the model dim
    NR = QR // 128  # 12 k-tiles over q_lora_rank
    NKR = KR // 128  # 4 k-tiles over kv_lora_rank

    with TileContext(nc) as tc, ExitStack() as ctx:
        persist = ctx.enter_context(tc.tile_pool(name="persist", bufs=1))

        # ---------------- constants ----------------
        ident = persist.tile([128, 128], F, name="c_ident")
        nc.sync.dma_start(ident[:], ident_d[:])
        triu = persist.tile([128, 128], BF, name="c_triu")
        nc.sync.dma_start(triu[:], triu_d[:])
        if32 = persist.tile([128, 32], F, name="c_if32")
        nc.sync.dma_start(if32[:], if32_d[:])
        if128 = persist.tile([128, 1], F, name="c_if128")
        nc.sync.dma_start(if128[:], if128_d[:])
        ones_f = persist.tile([128, 1], F, name="c_ones_f")
        nc.vector.memset(ones_f[:], 1.0)
        ones_fr = persist.tile([1, 128], F, name="c_ones_fr")
        nc.vector.memset(ones_fr[:], 1.0)
        ones_col = persist.tile([128, 1], R, name="c_ones_col")
        nc.vector.tensor_copy(ones_col[:], ones_f[:])
        ones_row = persist.tile([1, 128], R, name="c_ones_row")
        nc.vector.tensor_copy(ones_row[:], ones_fr[:])
        eps_t = persist.tile([128, 1], F, name="c_eps")
        nc.vector.memset(eps_t[:], EPS)
        sin_k = [persist.tile([128, 32], F, name=f"t_sink{st}") for st in range(2)]
        cos_k = [persist.tile([128, 32], F, name=f"t_cosk{st}") for st in range(2)]

        # ---------------- rope tables (early: overlaps initial DMA) ------
        # k_pe tables for the local 256 rows
        with tc.tile_pool(name="tabp", bufs=1) as tabp:
            posf_loc = tabp.tile([128, 2], F, name="posf_loc")
            pos_i_loc = tabp.tile([128, 2], I32, name="pos_i_loc")
            nc.sync.dma_start(
                pos_i_loc[:], pos_loc.ap().rearrange("(t p) -> p t", p=128)
            )
            nc.vector.tensor_copy(posf_loc[:], pos_i_loc[:])
            for st in range(2):
                nc.vector.tensor_scalar_mul(
                    sin_k[st][:], if32[:], posf_loc[:, st : st + 1]
                )
                nc.vector.tensor_scalar(
                    cos_k[st][:], sin_k[st][:], np.pi / 2.0, None, ALU.add
                )
                _emit_range_reduce(nc, tabp, sin_k[st][:], 32)
                _emit_range_reduce(nc, tabp, cos_k[st][:], 32)
                nc.scalar.activation(sin_k[st][:], sin_k[st][:], AF.Sin)
                nc.scalar.activation(cos_k[st][:], cos_k[st][:], AF.Sin)

        # q rope tables for the full sequence
        sin_q = persist.tile([128, S], F, name="t_sinq")
        cos_q = persist.tile([128, S], F, name="t_cosq")
        ssin_q = persist.tile([128, S], F, name="t_ssinq")
        sgn = persist.tile([128, 1], F, name="c_sgn")
        for b4 in range(4):
            nc.vector.memset(sgn[ds(32 * b4, 32), :], -1.0 if b4 % 2 == 0 else 1.0)
        with tc.tile_pool(name="tabq", bufs=1) as tabq, \
             tc.tile_pool(name="tabq_ps", bufs=2, space="PSUM") as tabq_ps:
            posf_row = tabq.tile([1, S], R, name="posf_row")
            pos_i_row = tabq.tile([1, S], I32, name="pos_i_row")
            nc.sync.dma_start(pos_i_row[:], pos_all[:])
            nc.vector.tensor_copy(posf_row[:], pos_i_row[:])
            for cchunk in range(4):
                bc = tabq_ps.tile([128, 512], F, tag="tab_ps", name=f"tabbc{cchunk}")
                nc.tensor.matmul(
                    bc[:], ones_row[:], posf_row[:, ts(cchunk, 512)],
                    start=True, stop=True,
                )
                nc.vector.tensor_scalar_mul(sin_q[:, ts(cchunk, 512)], bc[:], if128[:])
            nc.vector.tensor_scalar(cos_q[:], sin_q[:], np.pi / 2.0, None, ALU.add)
            _emit_range_reduce(nc, tabq, sin_q[:], S)
            _emit_range_reduce(nc, tabq, cos_q[:], S)
            nc.scalar.activation(sin_q[:], sin_q[:], AF.Sin)
            nc.scalar.activation(cos_q[:], cos_q[:], AF.Sin)
            nc.vector.tensor_scalar_mul(ssin_q[:], sin_q[:], sgn[:])

        # ---------------- phase 0: q_a / kv_a projections ----------------
        ctx0 = ExitStack()
        xtp = ctx0.enter_context(tc.tile_pool(name="xtp", bufs=1))
        wp0 = ctx0.enter_context(tc.tile_pool(name="wp0", bufs=3))
        p0 = ctx0.enter_context(tc.tile_pool(name="p0", bufs=2))

        xt = xtp.tile([128, NKT, SL], BF, name="xt")

        # --- kv_a first (so its AllGather hides under q_a compute) ---
        ctx0a = ExitStack()
        kv_ps = ctx0a.enter_context(tc.tile_pool(name="kv_ps", bufs=4, space="PSUM"))
        tr_ps = ctx0a.enter_context(tc.tile_pool(name="tr_ps", bufs=2, space="PSUM"))
        trk_ps = ctx0a.enter_context(tc.tile_pool(name="trk_ps", bufs=1, space="PSUM"))
        trsb = ctx0a.enter_context(tc.tile_pool(name="trsb", bufs=3))

        kvch = [[None] * 2 for _ in range(2)]
        for ch in range(2):
            for st in range(2):
                kvch[st][ch] = kv_ps.tile(
                    [128, 288], F, tag="kv_ps", name=f"kvps{st}_{ch}"
                )
        for ktg in range(4):
            # stream x in 8-ktile chunks so matmuls start before the full load
            nc.sync.dma_start(xt[:, ds(8 * ktg, 8), :], xT.ap()[:, ds(8 * ktg, 8), :])
            w = wp0.tile([128, 8, 576], BF, tag="kvw", name=f"kvw{ktg}")
            nc.sync.dma_start(w[:], kvawT.ap()[:, ds(8 * ktg, 8), :])
            for kk in range(8):
                kt = ktg * 8 + kk
                for st in range(2):
                    for ch in range(2):
                        nc.tensor.matmul(
                            kvch[st][ch][:],
                            xt[:, kt, ts(st, 128)],
                            w[:, kk, ts(ch, 288)],
                            start=(kt == 0), stop=(kt == NKT - 1),
                        )
        for st in range(2):
            acc0 = p0.tile([128, 1], F, tag="kvacc", name=f"kvacc0_{st}")
            acc1 = p0.tile([128, 1], F, tag="kvacc", name=f"kvacc1_{st}")
            scr = p0.tile([128, 288], F, tag="kvscr", name=f"kvscr{st}")
            nc.scalar.activation(scr[:], kvch[st][0][:], AF.Square, accum_out=acc0[:])
            nc.scalar.activation(
                scr[:, 0:224], kvch[st][1][:, 0:224], AF.Square, accum_out=acc1[:]
            )
            nc.vector.tensor_tensor(acc0[:], acc0[:], acc1[:], ALU.add)
            stdv = p0.tile([128, 1], F, tag="kvstd", name=f"kvstd{st}")
            nc.scalar.activation(stdv[:], acc0[:], AF.Sqrt, bias=eps_t[:], scale=1.0 / KR)
            rinv = p0.tile([128, 1], F, tag="kvrinv", name=f"kvrinv{st}")
            nc.vector.reciprocal(rinv[:], stdv[:])
            ckvn = p0.tile([128, KR], F, tag="ckvn", name=f"ckvn{st}")
            nc.vector.tensor_scalar_mul(ckvn[:, 0:288], kvch[st][0][:], rinv[:])
            nc.vector.tensor_scalar_mul(ckvn[:, 288:512], kvch[st][1][:, 0:224], rinv[:])
            # rope k_pe: cols 512:576 of kv_a = chunk1 cols 224:288, deinterleaved
            pe = kvch[st][1][:, 224:288].rearrange("p (d two) -> p two d", two=2)
            y1, y2 = pe[:, 0], pe[:, 1]
            kr_t = p0.tile([128, DR], F, tag="kr", name=f"kr{st}")
            t1 = p0.tile([128, 32], F, tag="krt1", name=f"krt1_{st}")
            t2 = p0.tile([128, 32], F, tag="krt2", name=f"krt2_{st}")
            nc.vector.tensor_tensor(t1[:], y1, cos_k[st][:], ALU.mult)
            nc.vector.tensor_tensor(t2[:], y2, sin_k[st][:], ALU.mult)
            nc.vector.tensor_tensor(kr_t[:, 0:32], t1[:], t2[:], ALU.subtract)
            nc.vector.tensor_tensor(t1[:], y2, cos_k[st][:], ALU.mult)
            nc.vector.tensor_tensor(t2[:], y1, sin_k[st][:], ALU.mult)
            nc.vector.tensor_tensor(kr_t[:, 32:64], t1[:], t2[:], ALU.add)
            for rt in range(KR // 128):
                tp = tr_ps.tile([128, 128], F, tag="tr", name=f"kvtr{st}_{rt}")
                nc.tensor.transpose(tp[:], ckvn[:, ts(rt, 128)], ident[:])
                sb_t = trsb.tile([128, 128], BF, tag="trsb", name=f"kvtrs{st}_{rt}")
                nc.any.tensor_copy(sb_t[:], tp[:])
                nc.gpsimd.dma_start(g1kv_src.ap()[ts(rt, 128), ts(st, 128)], sb_t[:])
            tpk = trk_ps.tile([64, 128], F, tag="trk", name=f"kvtrk{st}")
            nc.tensor.transpose(tpk[:], kr_t[:], ident[:])
            sb_k = trsb.tile([64, 128], BF, tag="trsbk", name=f"kvtrks{st}")
            nc.any.tensor_copy(sb_k[:], tpk[:])
            nc.gpsimd.dma_start(g1kv_src.ap()[KR : KR + DR, ts(st, 128)], sb_k[:])
        ctx0a.close()

        nc.gpsimd.collective_compute(
            "AllGather", ALU.bypass,
            ins=[g1kv_src.ap().opt()], outs=[g1kv.ap().opt()], replica_groups=RG,
        )

        # --- q_a (chunks of 512 cols) ---
        ctx0b = ExitStack()
        qa_ps = ctx0b.enter_context(tc.tile_pool(name="qa_ps", bufs=6, space="PSUM"))
        tr2_ps = ctx0b.enter_context(tc.tile_pool(name="tr2_ps", bufs=2, space="PSUM"))
        tr2sb = ctx0b.enter_context(tc.tile_pool(name="tr2sb", bufs=3))
        qch = [[None] * 3 for _ in range(2)]
        for ch in range(3):
            for st in range(2):
                qch[st][ch] = qa_ps.tile(
                    [128, 512], F, tag="qa_ps", name=f"qaps{st}_{ch}"
                )
        for ch in range(3):
            for ktg in range(NKT // 8):
                w = wp0.tile([128, 8, 512], BF, tag="qaw", name=f"qaw{ch}_{ktg}")
                nc.scalar.dma_start(w[:], qawT.ap()[ch, :, ds(8 * ktg, 8), :])
                for kk in range(8):
                    kt = ktg * 8 + kk
                    for st in range(2):
                        nc.tensor.matmul(
                            qch[st][ch][:], xt[:, kt, ts(st, 128)], w[:, kk],
                            start=(kt == 0), stop=(kt == NKT - 1),
                        )
        for st in range(2):
            accs = []
            scr = p0.tile([128, 512], F, tag="qascr", name=f"qascr{st}")
            for ch in range(3):
                a = p0.tile([128, 1], F, tag="qaacc", name=f"qaacc{st}_{ch}")
                nc.scalar.activation(scr[:], qch[st][ch][:], AF.Square, accum_out=a[:])
                accs.append(a)
            nc.vector.tensor_tensor(accs[0][:], accs[0][:], accs[1][:], ALU.add)
            nc.vector.tensor_tensor(accs[0][:], accs[0][:], accs[2][:], ALU.add)
            stdv = p0.tile([128, 1], F, tag="qastd", name=f"qastd{st}")
            nc.scalar.activation(stdv[:], accs[0][:], AF.Sqrt, bias=eps_t[:], scale=1.0 / QR)
            rinv = p0.tile([128, 1], F, tag="qarinv", name=f"qarinv{st}")
            nc.vector.reciprocal(rinv[:], stdv[:])
            qn = p0.tile([128, QR], F, tag="qn", name=f"qn{st}")
            for ch in range(3):
                nc.vector.tensor_scalar_mul(qn[:, ts(ch, 512)], qch[st][ch][:], rinv[:])
            for rt in range(NR):
                tp = tr2_ps.tile([128, 128], F, tag="tr2", name=f"qtr{st}_{rt}")
                nc.tensor.transpose(tp[:], qn[:, ts(rt, 128)], ident[:])
                sb_t = tr2sb.tile([128, 128], BF, tag="tr2sb", name=f"qtrs{st}_{rt}")
                nc.any.tensor_copy(sb_t[:], tp[:])
                nc.gpsimd.dma_start(g1q_src.ap()[ts(rt, 128), ts(st, 128)], sb_t[:])
        ctx0b.close()
        ctx0.close()

        nc.gpsimd.collective_compute(
            "AllGather", ALU.bypass,
            ins=[g1q_src.ap().opt()], outs=[g1q.ap().opt()], replica_groups=RG,
        )

        # ---------------- shared latents on-chip ----------------
        ctx_att = ExitStack()
        attb = ctx_att.enter_context(tc.tile_pool(name="attb", bufs=1))

        kpe_rep = attb.tile([128, S], BF, name="kpe_rep")
        for half in range(2):
            nc.sync.dma_start(
                kpe_rep[ds(64 * half, 64), :].rearrange("p (c s) -> p c s", c=NCORES),
                g1kv.ap()[:, KR : KR + DR, :].rearrange("c p s -> p c s"),
            )

        qnope = [attb.tile([128, S], BF, name=f"qnope{h}") for h in range(HPC)]
        qfpe = [attb.tile([128, S], BF, name=f"qfpe{p}") for p in range(2)]
        v_sb = attb.tile([128, S // 128, 512], BF, name="v_sb")
        kn_all = [attb.tile([128, S], BF, name=f"kn{h}") for h in range(HPC)]

        # ---------------- phase 1: kv decompression (hides q AllGather) --
        ctxd = ExitStack()
        ckvp = ctxd.enter_context(tc.tile_pool(name="ckvp", bufs=1))
        dec_ps = ctxd.enter_context(tc.tile_pool(name="dec_ps", bufs=2, space="PSUM"))
        ckv_t = []
        for r in range(NKR):
            t = ckvp.tile([128, S], BF, name=f"ckv{r}")
            nc.sync.dma_start(
                t[:].rearrange("p (c s) -> p c s", c=NCORES),
                g1kv.ap()[:, ts(r, 128), :].rearrange("c p s -> p c s"),
            )
            ckv_t.append(t)
        kvbv, kvbn = [], []
        for r in range(NKR):
            tv = ckvp.tile([128, 512], BF, name=f"kvbv{r}")
            nc.sync.dma_start(tv[:], kvbwT.ap()[ts(r, 128), 512:1024])
            kvbv.append(tv)
            tn = ckvp.tile([128, 512], BF, name=f"kvbn{r}")
            nc.sync.dma_start(tn[:], kvbwT.ap()[ts(r, 128), 0:512])
            kvbn.append(tn)

        for st in range(S // 128):
            ps = dec_ps.tile([128, 512], F, tag="dec", name=f"vps{st}")
            for r in range(NKR):
                nc.tensor.matmul(
                    ps[:], ckv_t[r][:, ts(st, 128)], kvbv[r][:],
                    start=(r == 0), stop=(r == NKR - 1),
                )
            nc.any.tensor_copy(v_sb[:, st, :], ps[:])
        for h in range(HPC):
            for sb in range(4):
                ps = dec_ps.tile([128, 512], F, tag="dec", name=f"knps{h}_{sb}")
                for r in range(NKR):
                    nc.tensor.matmul(
                        ps[:], kvbn[r][:, ts(h, 128)], ckv_t[r][:, ts(sb, 512)],
                        start=(r == 0), stop=(r == NKR - 1),
                    )
                nc.any.tensor_copy(kn_all[h][:, ts(sb, 512)], ps[:])
        ctxd.close()

        # ---------------- phase 2: q_b projection (+ q rope) ----------------
        ctx2 = ExitStack()
        qrp = ctx2.enter_context(tc.tile_pool(name="qrp", bufs=13))
        qbwp = ctx2.enter_context(tc.tile_pool(name="qbwp", bufs=1))
        ropep = ctx2.enter_context(tc.tile_pool(name="ropep", bufs=2))
        qb_ps = ctx2.enter_context(tc.tile_pool(name="qb_ps", bufs=3, space="PSUM"))

        # q_b weights resident in SBUF, loaded once (3 MB)
        qbw = []
        for r in range(NR):
            w = qbwp.tile([128, 768], BF, name=f"qbw{r}")
            nc.scalar.dma_start(w[:], qbwT.ap()[ts(r, 128), :])
            qbw.append(w)

        JORD = [0, 1, 2, 3, 4, 5]
        for sb in range(4):
            qr_tiles = []
            for r in range(NR):
                t = qrp.tile([128, 512], BF, tag="qr", name=f"qr{sb}_{r}")
                nc.sync.dma_start(
                    t[:].rearrange("p (c s) -> p c s", c=2),
                    g1q.ap()[2 * sb : 2 * sb + 2, ts(r, 128), :]
                    .rearrange("c p s -> p c s"),
                )
                qr_tiles.append(t)
            ps_of = {}
            for j in JORD:
                psj = qb_ps.tile([128, 512], F, tag="qb_ps", name=f"qbps{sb}_{j}")
                ps_of[j] = psj
                for r in range(NR):
                    nc.tensor.matmul(
                        psj[:], qbw[r][:, ts(j, 128)], qr_tiles[r][:],
                        start=(r == 0), stop=(r == NR - 1),
                    )
                if j < 4:
                    nc.any.tensor_copy(qnope[j][:, ts(sb, 512)], psj[:])
                else:
                    p = j - 4
                    t1 = ropep.tile([128, 512], F, tag="rope1", name=f"rp1_{sb}_{p}")
                    t2 = ropep.tile([128, 512], F, tag="rope2", name=f"rp2_{sb}_{p}")
                    nc.vector.tensor_tensor(
                        t1[:], psj[:], cos_q[:, ts(sb, 512)], ALU.mult
                    )
                    for o in (0, 64):
                        nc.vector.tensor_tensor(
                            t2[ds(o, 32), :], psj[ds(o + 32, 32), :],
                            ssin_q[ds(o, 32), ts(sb, 512)], ALU.mult,
                        )
                        nc.vector.tensor_tensor(
                            t2[ds(o + 32, 32), :], psj[ds(o, 32), :],
                            ssin_q[ds(o + 32, 32), ts(sb, 512)], ALU.mult,
                        )
                    nc.vector.tensor_tensor(
                        qfpe[p][:, ts(sb, 512)], t1[:], t2[:], ALU.add
                    )
        ctx2.close()

        # ---------------- phase 3: attention + per-head AllToAll ---------
        ctx3 = ExitStack()
        probp = ctx3.enter_context(tc.tile_pool(name="probp", bufs=6))
        invp = ctx3.enter_context(tc.tile_pool(name="invp", bufs=2))
        psump = ctx3.enter_context(tc.tile_pool(name="psump", bufs=4))
        attp = ctx3.enter_context(tc.tile_pool(name="attp", bufs=2))
        denp = ctx3.enter_context(tc.tile_pool(name="denp", bufs=2))
        ctx3p = ExitStack()
        sc_ps = ctx3p.enter_context(tc.tile_pool(name="sc_ps", bufs=2, space="PSUM"))
        av_ps = ctx3p.enter_context(tc.tile_pool(name="av_ps", bufs=4, space="PSUM"))
        tail_ps = ctx3p.enter_context(tc.tile_pool(name="tail_ps", bufs=2, space="PSUM"))

        # The four q-blocks of a head run phase-staggered so the PE always
        # has 2-4 independent score->exp->AV chains in flight (keeps the
        # activity clock-gate warm).
        QOFF = {3: 0, 2: 2, 1: 4, 0: 6}
        for h in range(HPC):
            kn = kn_all[h]
            pe_rhs = qfpe[h // 2][ds(64 * (h % 2), 64), :]
            pe_lhs = kpe_rep[ds(64 * (h % 2), 64), :]
            avt_of, psum_of = {}, {}
            for s in range(16):
                for qb in (3, 2, 1, 0):
                    kt = s - QOFF[qb]
                    nkt = 4 * (qb + 1)
                    if not (0 <= kt < nkt):
                        continue
                    if kt == 0:
                        avt_of[qb] = av_ps.tile(
                            [128, 512], F, tag="av", name=f"av{h}_{qb}"
                        )
                        psum_of[qb] = psump.tile(
                            [128, 512], R, tag="psum", name=f"psum{h}_{qb}"
                        )
                    avt, psum = avt_of[qb], psum_of[qb]
                    trim = max(0, 128 * (kt - 4 * qb))
                    qsl = ds(512 * qb + trim, 512 - trim)
                    sct = sc_ps.tile([128, 512], F, tag="sc", name=f"sc{h}{qb}_{kt}")
                    nc.tensor.matmul(
                        sct[:, trim:512], kn[:, ts(kt, 128)], qnope[h][:, qsl],
                        start=True, stop=False,
                    )
                    nc.tensor.matmul(
                        sct[:, trim:512], pe_lhs[:, ts(kt, 128)], pe_rhs[:, qsl],
                        start=False, stop=True,
                    )
                    prob = probp.tile([128, 512], BF, tag="prob", name=f"pr{h}{qb}_{kt}")
                    nc.scalar.activation(prob[:, trim:512], sct[:, trim:512], AF.Exp)
                    if kt >= 4 * qb:
                        nc.vector.tensor_tensor(
                            prob[:, trim : trim + 128],
                            prob[:, trim : trim + 128],
                            triu[:],
                            ALU.mult,
                        )
                    nc.tensor.matmul(
                        avt[:, trim:512], v_sb[:, kt, ts(h, 128)], prob[:, trim:512],
                        start=(kt == 0), stop=(kt == nkt - 1),
                    )
                    if kt == 0:
                        nc.vector.tensor_copy(psum[:], prob[:])
                    else:
                        nc.vector.tensor_tensor(
                            psum[:, trim:512], psum[:, trim:512],
                            prob[:, trim:512], ALU.add,
                        )
                    if kt == nkt - 1:
                        # att = avt * exp(-ln(den)): Ln/Exp share one ACT
                        # table set, so no reciprocal (8 cyc/elem on DVE)
                        # and no table thrash.
                        dent = tail_ps.tile([1, 512], F, tag="tail", name=f"den{h}_{qb}")
                        nc.tensor.matmul(
                            dent[:], ones_col[:], psum[:], start=True, stop=True
                        )
                        lden = denp.tile([1, 512], R, tag="lden", name=f"lden{h}_{qb}")
                        nc.scalar.activation(lden[:], dent[:], AF.Ln)
                        bct = tail_ps.tile([128, 512], F, tag="tail", name=f"bc{h}_{qb}")
                        nc.tensor.matmul(
                            bct[:], ones_row[:], lden[:], start=True, stop=True
                        )
                        einv = invp.tile([128, 512], F, tag="einv", name=f"einv{h}_{qb}")
                        nc.scalar.activation(einv[:], bct[:], AF.Exp, scale=-1.0)
                        att = attp.tile([128, 512], BF, tag="att", name=f"att{h}_{qb}")
                        nc.vector.tensor_tensor(att[:], avt[:], einv[:], ALU.mult)
                        nc.gpsimd.dma_start(a2a_src[h].ap()[qb], att[:])
                        nc.gpsimd.dma_start(a2a_src[h].ap()[qb + 4], att[:])

            nc.gpsimd.collective_compute(
                "AllToAll", ALU.bypass,
                ins=[a2a_src[h].ap().opt()], outs=[a2a_out[h].ap().opt()],
                replica_groups=RG,
            )

        ctx3p.close()

        # ---------------- phase 4: o_proj (my H-half x my q-block) -------
        ctx5 = ExitStack()
        owp = ctx5.enter_context(tc.tile_pool(name="owp", bufs=3))
        a2ap = ctx5.enter_context(tc.tile_pool(name="a2ap", bufs=1))
        osb = ctx5.enter_context(tc.tile_pool(name="osb", bufs=2))
        o_ps = ctx5.enter_context(tc.tile_pool(name="o_ps", bufs=2, space="PSUM"))

        # attention outputs for my q-block: nd chunk (h*8 + i) = head 4i+h.
        # Head 3's loads are emitted after the G1 passes so they don't
        # head-of-line block the o_proj weight stream behind AllToAll #3.
        a2a_sb = []
        for h in range(HPC - 1):
            for i in range(NCORES):
                t = a2ap.tile([128, QBLK], BF, name=f"a2a_{h}_{i}")
                nc.sync.dma_start(t[:], a2a_out[h].ap()[i])
                a2a_sb.append(t)

        # G1 (heads from A2A 0-2) runs while the last AllToAll is in flight;
        # G2 (A2A 3's 8 chunks) accumulates into DRAM afterwards.
        for ht in range(16):
            ow_t = owp.tile([128, 24, 128], BF, tag="ow", name=f"ow{ht}")
            nc.sync.dma_start(
                ow_t[:], owT.ap()[ht, 0:3072, :].rearrange("(nd p) j -> p nd j", p=128)
            )
            ps = o_ps.tile([128, QBLK], F, tag="o", name=f"ops{ht}")
            for nd in range(24):
                nc.tensor.matmul(
                    ps[:], ow_t[:, nd, :], a2a_sb[nd][:],
                    start=(nd == 0), stop=(nd == 23),
                )
            ost = osb.tile([128, QBLK], F, tag="osb", name=f"osb{ht}")
            nc.any.tensor_copy(ost[:], ps[:])
            nc.gpsimd.dma_start(out_d.ap()[ts(ht, 128), :], ost[:])
        for h in (HPC - 1,):
            for i in range(NCORES):
                t = a2ap.tile([128, QBLK], BF, name=f"a2a_{h}_{i}")
                nc.sync.dma_start(t[:], a2a_out[h].ap()[i])
                a2a_sb.append(t)
        for ht in range(16):
            ow_t = owp.tile([128, 8, 128], BF, tag="ow2", name=f"ow2_{ht}")
            nc.sync.dma_start(
                ow_t[:],
                owT.ap()[ht, 3072:4096, :].rearrange("(nd p) j -> p nd j", p=128),
            )
            ps = o_ps.tile([128, QBLK], F, tag="o", name=f"ops2_{ht}")
            for nd in range(8):
                nc.tensor.matmul(
                    ps[:], ow_t[:, nd, :], a2a_sb[24 + nd][:],
                    start=(nd == 0), stop=(nd == 7),
                )
            ost = osb.tile([128, QBLK], F, tag="osb", name=f"osb2_{ht}")
            nc.any.tensor_copy(ost[:], ps[:])
            nc.gpsimd.dma_start(
                out_d.ap()[ts(ht, 128), :], ost[:], accum_op=ALU.add
            )
        ctx5.close()
        ctx3.close()
        ctx_att.close()

    nc.compile()
    return nc


def _get_nc():
    global _CACHED_NC
    if _CACHED_NC is None:
        _CACHED_NC = _build_program()
    return _CACHED_NC


def _prep_inputs(hidden_states, position_ids, q_a_w, q_a_ln_w, q_b_w, kv_a_w,
                 kv_a_ln_w, kv_b_w, o_w):
    hidden_states = np.asarray(hidden_states, dtype=np.float32)
    position_ids = np.asarray(position_ids, dtype=np.int32)
    q_a_w = np.asarray(q_a_w, dtype=np.float32)
    q_a_ln_w = np.asarray(q_a_ln_w, dtype=np.float32)
    q_b_w = np.asarray(q_b_w, dtype=np.float32)
    kv_a_w = np.asarray(kv_a_w, dtype=np.float32)
    kv_a_ln_w = np.asarray(kv_a_ln_w, dtype=np.float32)
    kv_b_w = np.asarray(kv_b_w, dtype=np.float32)
    o_w = np.asarray(o_w, dtype=np.float32)

    bf = ml_dtypes.bfloat16
    xT_full = np.ascontiguousarray(hidden_states[0].T)          # (H, S)
    qawT = np.ascontiguousarray(q_a_w.T)                         # (H, QR)
    kvawT = np.ascontiguousarray(kv_a_w.T)                       # (H, KR+DR)
    # pre-rearranged for contiguous SBUF-layout DMA: (128, H/128, cols)
    kvawT_c = np.ascontiguousarray(
        kvawT.reshape(H // 128, 128, KR + DR).transpose(1, 0, 2)
    )
    qawT_c = np.ascontiguousarray(
        qawT.reshape(H // 128, 128, QR).transpose(1, 0, 2)      # (128, 32, QR)
        .reshape(128, H // 128, 3, 512).transpose(2, 0, 1, 3)   # (3, 128, 32, 512)
    )
    ident = np.eye(128, dtype=np.float32)
    triu = np.triu(np.ones((128, 128), dtype=np.float32)).astype(bf)
    inv_freq = (1.0 / (THETA ** (np.arange(0, DR, 2) / DR))).astype(np.float32)
    if32 = np.tile(inv_freq[None, :], (128, 1)).astype(np.float32)
    if128 = np.tile(inv_freq, 4)[:, None].astype(np.float32)
    owT_full = np.ascontiguousarray(o_w.T)                       # (N*DV, H)

    in_maps = []
    for c in range(NCORES):
        heads = slice(HPC * c, HPC * (c + 1))
        qb = q_b_w.reshape(N_HEADS, QD, QR)[heads]               # (4, 192, QR)
        nope = qb[:, :DN, :].reshape(HPC * DN, QR)
        pe = qb[:, DN:, :]
        pe_d = np.concatenate([pe[:, 0::2, :], pe[:, 1::2, :]], axis=1)  # (4,64,QR)
        cols = np.concatenate(
            [nope, pe_d.reshape(HPC * DR, QR)], axis=0
        )                                                        # (768, QR)
        qbwT_c = np.ascontiguousarray((cols * (SCALE * q_a_ln_w[None, :])).T)

        kvb = kv_b_w.reshape(N_HEADS, DN + DV, KR)[heads]
        kcols = np.concatenate(
            [kvb[:, :DN, :].reshape(HPC * DN, KR),
             kvb[:, DN:, :].reshape(HPC * DV, KR)],
            axis=0,
        )                                                        # (1024, KR)
        kvbwT_c = np.ascontiguousarray((kcols * kv_a_ln_w[None, :]).T)

        # o_proj weights: rows ordered (h_loc, src_rank) -> head 4*i + h_loc,
        # column slice = this core's H-half, chunked per 128-col ht pass.
        hhalf = c // 4
        row_order = np.empty((HPC * NCORES,), dtype=np.int64)
        for h_loc in range(HPC):
            for i in range(NCORES):
                row_order[h_loc * NCORES + i] = 4 * i + h_loc
        owT_rows = owT_full.reshape(N_HEADS, DV, H)[row_order].reshape(
            N_HEADS * DV, H
        )[:, HHALF * hhalf : HHALF * (hhalf + 1)]                # (4096, 2048)
        owT_c = np.ascontiguousarray(
            owT_rows.reshape(N_HEADS * DV, 16, 128).transpose(1, 0, 2)
        )                                                        # (16, 4096, 128)

        xT_c = np.ascontiguousarray(
            xT_full[:, SL * c : SL * (c + 1)].reshape(H // 128, 128, SL)
            .transpose(1, 0, 2)
        )                                                        # (128, 32, SL)
        in_maps.append(
            {
                "xT": xT_c.astype(bf),
                "qawT": qawT_c.astype(bf),
                "kvawT": kvawT_c.astype(bf),
                "qbwT": qbwT_c.astype(bf),
                "kvbwT": kvbwT_c.astype(bf),
                "owT": owT_c.astype(bf),
                "pos_all": np.ascontiguousarray(position_ids.reshape(1, S)),
                "pos_loc": np.ascontiguousarray(
                    position_ids.reshape(-1)[SL * c : SL * (c + 1)]
                ),
                "ident": ident,
                "triu": triu,
                "if32": if32,
                "if128": if128,
            }
        )
    return in_maps


def kernel(**inputs):
    global LAST_RESULT
    nc = _get_nc()
    in_maps = _prep_inputs(**inputs)
    res = run_bass_kernel_spmd(nc, in_maps, list(range(NCORES)))
    LAST_RESULT = res
    outT = np.empty((H, S), dtype=np.float32)
    for c in range(NCORES):
        hhalf, qblk = c // 4, c % 4
        outT[HHALF * hhalf : HHALF * (hhalf + 1),
             QBLK * qblk : QBLK * (qblk + 1)] = res.results[c]["out"]
    return outT.T[None].astype(np.float32)


# revision 22
# speedup vs baseline: 1.0070x; 1.0070x over previous
"""DeepSeekV3 MLA attention prefill kernel for 8 Trainium2 NeuronCores.

Sharding: sequence-parallel low-rank input projections (q_a / kv_a),
AllGather of the shared latents (kv first, hidden under q_a compute; q AG
hidden under kv decompression), tensor-parallel over heads (4 heads/core)
for q_b / kv_b decompression and attention, per-head AllToAll to
redistribute attention outputs seq-wise (8x less wire than AllGather), and
a 4x2 (seq-block x H-half) sharded o_proj with streamed weights.

Matmuls: projections bf16, score-nope fp32r, score-pe bf16, AV bf16.
"""

import sys

sys.path.insert(0, "/opt/trn_rl_repo")

import numpy as np
import ml_dtypes

import concourse.bass as bass  # noqa: F401
import concourse.mybir as mybir
from concourse import bacc
import functools as _ft

# Route every Exp to natural_log_exp_and_others (which genuinely contains
# Exp) so the attention's Ln/Exp mix stays in ONE ACT table set - otherwise
# the table-load pass alternates sets per softmax tail (~2.7us each).
_orig_get_tables = bacc.get_activation_tables


@_ft.cache
def _exp_unified_tables(module_arch):
    AFT = mybir.ActivationFunctionType
    out = {}
    for name, fns in _orig_get_tables(module_arch).items():
        fns = set(fns)
        if name != "natural_log_exp_and_others":
            fns.discard(AFT.Exp)
        out[name] = fns
    return out


bacc.get_activation_tables = _exp_unified_tables
from concourse.bass import ds, ts
from concourse.tile import TileContext
from concourse.bass_utils import run_bass_kernel_spmd
from contextlib import ExitStack

F = mybir.dt.float32
BF = mybir.dt.bfloat16
R = mybir.dt.float32r
I32 = mybir.dt.int32
AF = mybir.ActivationFunctionType
ALU = mybir.AluOpType

NCORES = 8
B, S, H = 1, 2048, 4096
N_HEADS = 32
HPC = N_HEADS // NCORES          # heads per core = 4
SL = S // NCORES                 # sequence rows per core = 256
QR, KR = 1536, 512
DR, DN, DV = 64, 128, 128
QD = DN + DR                     # 192
SCALE = QD ** -0.5
EPS = 1e-6
THETA = 10000.0
TWO_PI = float(2.0 * np.pi)
QBLK = 512                       # o_proj q-block per core (4-way over seq)
HHALF = H // 2                   # o_proj H columns per core (2-way)

LAST_RESULT = None               # test harness reads exec_time_ns from here
_CACHED_NC = None
_UID = [0]


def _uid():
    _UID[0] += 1
    return _UID[0]


def _emit_range_reduce(nc, pool, t_ap, width):
    """In-place wrap t_ap (f32, [128, width]) to [-pi, pi]. f32->i32 copy
    rounds to nearest (verified on HW)."""
    tn = pool.tile([128, width], F, tag=f"rr_f_{width}", name=f"rrf{_uid()}")
    ti = pool.tile([128, width], I32, tag=f"rr_i_{width}", name=f"rri{_uid()}")
    nc.vector.tensor_scalar_mul(tn[:], t_ap, 1.0 / TWO_PI)
    nc.vector.tensor_copy(ti[:], tn[:])
    nc.vector.tensor_copy(tn[:], ti[:])
    nc.vector.tensor_scalar_mul(tn[:], tn[:], -TWO_PI)
    nc.vector.tensor_tensor(t_ap, t_ap, tn[:], ALU.add)


def _build_program():
    nc = bacc.Bacc(None, target_bir_lowering=False, num_devices=NCORES)

    # ---------------- DRAM declarations ----------------
    xT = nc.dram_tensor("xT", [128, H // 128, SL], BF, kind="ExternalInput")
    qawT = nc.dram_tensor("qawT", [3, 128, H // 128, 512], BF, kind="ExternalInput")
    kvawT = nc.dram_tensor("kvawT", [128, H // 128, KR + DR], BF, kind="ExternalInput")
    qbwT = nc.dram_tensor("qbwT", [QR, 768], BF, kind="ExternalInput")
    kvbwT = nc.dram_tensor("kvbwT", [KR, 1024], BF, kind="ExternalInput")
    owT = nc.dram_tensor("owT", [16, 4096, 128], BF, kind="ExternalInput")
    pos_all = nc.dram_tensor("pos_all", [1, S], I32, kind="ExternalInput")
    pos_loc = nc.dram_tensor("pos_loc", [SL], I32, kind="ExternalInput")
    ident_d = nc.dram_tensor("ident", [128, 128], F, kind="ExternalInput")
    triu_d = nc.dram_tensor("triu", [128, 128], BF, kind="ExternalInput")
    if32_d = nc.dram_tensor("if32", [128, 32], F, kind="ExternalInput")
    if128_d = nc.dram_tensor("if128", [128, 1], F, kind="ExternalInput")
    out_d = nc.dram_tensor("out", [HHALF, QBLK], F, kind="ExternalOutput")

    g1kv_src = nc.dram_tensor("g1kv_src", [KR + DR, SL], BF)
    g1kv = nc.dram_tensor("g1kv", [NCORES, KR + DR, SL], BF, addr_space="Shared")
    g1q_src = nc.dram_tensor("g1q_src", [QR, SL], BF)
    g1q = nc.dram_tensor("g1q", [NCORES, QR, SL], BF, addr_space="Shared")
    # per-head AllToAll buffers: shard j holds this core's head-h attention
    # output for q-range (j % 4); shards j and j+4 are identical copies so
    # cores j and j+4 (the two H-halves) both receive that q-range.
    a2a_src = [nc.dram_tensor(f"a2as{h}", [NCORES, DV, QBLK], BF) for h in range(HPC)]
    a2a_out = [
        nc.dram_tensor(f"a2ao{h}", [NCORES, DV, QBLK], BF) for h in range(HPC)
    ]
    RG = [list(range(NCORES))]
    NKT = H // 128  # 32 k-tiles over the model dim
    NR = QR // 128  # 12 k-tiles over q_lora_rank
    NKR = KR // 128  # 4 k-tiles over kv_lora_rank

    with TileContext(nc) as tc, ExitStack() as ctx:
        persist = ctx.enter_context(tc.tile_pool(name="persist", bufs=1))

        # ---------------- constants ----------------
        ident = persist.tile([128, 128], F, name="c_ident")
        nc.sync.dma_start(ident[:], ident_d[:])
        triu = persist.tile([128, 128], BF, name="c_triu")
        nc.sync.dma_start(triu[:], triu_d[:])
        if32 = persist.tile([128, 32], F, name="c_if32")
        nc.sync.dma_start(if32[:], if32_d[:])
        if128 = persist.tile([128, 1], F, name="c_if128")
        nc.sync.dma_start(if128[:], if128_d[:])
        ones_f = persist.tile([128, 1], F, name="c_ones_f")
        nc.vector.memset(ones_f[:], 1.0)
        ones_fr = persist.tile([1, 128], F, name="c_ones_fr")
        nc.vector.memset(ones_fr[:], 1.0)
        ones_col = persist.tile([128, 1], R, name="c_ones_col")
        nc.vector.tensor_copy(ones_col[:], ones_f[:])
        ones_row = persist.tile([1, 128], R, name="c_ones_row")
        nc.vector.tensor_copy(ones_row[:], ones_fr[:])
        eps_t = persist.tile([128, 1], F, name="c_eps")
        nc.vector.memset(eps_t[:], EPS)
        sin_k = [persist.tile([128, 32], F, name=f"t_sink{st}") for st in range(2)]
        cos_k = [persist.tile([128, 32], F, name=f"t_cosk{st}") for st in range(2)]

        # ---------------- rope tables (early: overlaps initial DMA) ------
        # k_pe tables for the local 256 rows
        with tc.tile_pool(name="tabp", bufs=1) as tabp:
            posf_loc = tabp.tile([128, 2], F, name="posf_loc")
            pos_i_loc = tabp.tile([128, 2], I32, name="pos_i_loc")
            nc.sync.dma_start(
                pos_i_loc[:], pos_loc.ap().rearrange("(t p) -> p t", p=128)
            )
            nc.vector.tensor_copy(posf_loc[:], pos_i_loc[:])
            for st in range(2):
                nc.vector.tensor_scalar_mul(
                    sin_k[st][:], if32[:], posf_loc[:, st : st + 1]
                )
                nc.vector.tensor_scalar(
                    cos_k[st][:], sin_k[st][:], np.pi / 2.0, None, ALU.add
                )
                _emit_range_reduce(nc, tabp, sin_k[st][:], 32)
                _emit_range_reduce(nc, tabp, cos_k[st][:], 32)
                nc.scalar.activation(sin_k[st][:], sin_k[st][:], AF.Sin)
                nc.scalar.activation(cos_k[st][:], cos_k[st][:], AF.Sin)

        # q rope tables for the full sequence
        sin_q = persist.tile([128, S], F, name="t_sinq")
        cos_q = persist.tile([128, S], F, name="t_cosq")
        ssin_q = persist.tile([128, S], F, name="t_ssinq")
        sgn = persist.tile([128, 1], F, name="c_sgn")
        for b4 in range(4):
            nc.vector.memset(sgn[ds(32 * b4, 32), :], -1.0 if b4 % 2 == 0 else 1.0)
        with tc.tile_pool(name="tabq", bufs=1) as tabq, \
             tc.tile_pool(name="tabq_ps", bufs=2, space="PSUM") as tabq_ps:
            posf_row = tabq.tile([1, S], R, name="posf_row")
            pos_i_row = tabq.tile([1, S], I32, name="pos_i_row")
            nc.sync.dma_start(pos_i_row[:], pos_all[:])
            nc.vector.tensor_copy(posf_row[:], pos_i_row[:])
            for cchunk in range(4):
                bc = tabq_ps.tile([128, 512], F, tag="tab_ps", name=f"tabbc{cchunk}")
                nc.tensor.matmul(
                    bc[:], ones_row[:], posf_row[:, ts(cchunk, 512)],
                    start=True, stop=True,
                )
                nc.vector.tensor_scalar_mul(sin_q[:, ts(cchunk, 512)], bc[:], if128[:])
            nc.vector.tensor_scalar(cos_q[:], sin_q[:], np.pi / 2.0, None, ALU.add)
            _emit_range_reduce(nc, tabq, sin_q[:], S)
            _emit_range_reduce(nc, tabq, cos_q[:], S)
            nc.scalar.activation(sin_q[:], sin_q[:], AF.Sin)
            nc.scalar.activation(cos_q[:], cos_q[:], AF.Sin)
            nc.vector.tensor_scalar_mul(ssin_q[:], sin_q[:], sgn[:])

        # ---------------- phase 0: q_a / kv_a projections ----------------
        ctx0 = ExitStack()
        xtp = ctx0.enter_context(tc.tile_pool(name="xtp", bufs=1))
        wp0 = ctx0.enter_context(tc.tile_pool(name="wp0", bufs=3))
        p0 = ctx0.enter_context(tc.tile_pool(name="p0", bufs=2))

        xt = xtp.tile([128, NKT, SL], BF, name="xt")

        # --- kv_a first (so its AllGather hides under q_a compute) ---
        ctx0a = ExitStack()
        kv_ps = ctx0a.enter_context(tc.tile_pool(name="kv_ps", bufs=4, space="PSUM"))
        tr_ps = ctx0a.enter_context(tc.tile_pool(name="tr_ps", bufs=2, space="PSUM"))
        trk_ps = ctx0a.enter_context(tc.tile_pool(name="trk_ps", bufs=1, space="PSUM"))
        trsb = ctx0a.enter_context(tc.tile_pool(name="trsb", bufs=3))

        kvch = [[None] * 2 for _ in range(2)]
        for ch in range(2):
            for st in range(2):
                kvch[st][ch] = kv_ps.tile(
                    [128, 288], F, tag="kv_ps", name=f"kvps{st}_{ch}"
                )
        for ktg in range(4):
            # stream x in 8-ktile chunks so matmuls start before the full load
            nc.sync.dma_start(xt[:, ds(8 * ktg, 8), :], xT.ap()[:, ds(8 * ktg, 8), :])
            w = wp0.tile([128, 8, 576], BF, tag="kvw", name=f"kvw{ktg}")
            nc.sync.dma_start(w[:], kvawT.ap()[:, ds(8 * ktg, 8), :])
            for kk in range(8):
                kt = ktg * 8 + kk
                for st in range(2):
                    for ch in range(2):
                        nc.tensor.matmul(
                            kvch[st][ch][:],
                            xt[:, kt, ts(st, 128)],
                            w[:, kk, ts(ch, 288)],
                            start=(kt == 0), stop=(kt == NKT - 1),
                        )
        for st in range(2):
            acc0 = p0.tile([128, 1], F, tag="kvacc", name=f"kvacc0_{st}")
            acc1 = p0.tile([128, 1], F, tag="kvacc", name=f"kvacc1_{st}")
            scr = p0.tile([128, 288], F, tag="kvscr", name=f"kvscr{st}")
            nc.scalar.activation(scr[:], kvch[st][0][:], AF.Square, accum_out=acc0[:])
            nc.scalar.activation(
                scr[:, 0:224], kvch[st][1][:, 0:224], AF.Square, accum_out=acc1[:]
            )
            nc.vector.tensor_tensor(acc0[:], acc0[:], acc1[:], ALU.add)
            stdv = p0.tile([128, 1], F, tag="kvstd", name=f"kvstd{st}")
            nc.scalar.activation(stdv[:], acc0[:], AF.Sqrt, bias=eps_t[:], scale=1.0 / KR)
            rinv = p0.tile([128, 1], F, tag="kvrinv", name=f"kvrinv{st}")
            nc.vector.reciprocal(rinv[:], stdv[:])
            ckvn = p0.tile([128, KR], F, tag="ckvn", name=f"ckvn{st}")
            nc.vector.tensor_scalar_mul(ckvn[:, 0:288], kvch[st][0][:], rinv[:])
            nc.vector.tensor_scalar_mul(ckvn[:, 288:512], kvch[st][1][:, 0:224], rinv[:])
            # rope k_pe: cols 512:576 of kv_a = chunk1 cols 224:288, deinterleaved
            pe = kvch[st][1][:, 224:288].rearrange("p (d two) -> p two d", two=2)
            y1, y2 = pe[:, 0], pe[:, 1]
            kr_t = p0.tile([128, DR], F, tag="kr", name=f"kr{st}")
            t1 = p0.tile([128, 32], F, tag="krt1", name=f"krt1_{st}")
            t2 = p0.tile([128, 32], F, tag="krt2", name=f"krt2_{st}")
            nc.vector.tensor_tensor(t1[:], y1, cos_k[st][:], ALU.mult)
            nc.vector.tensor_tensor(t2[:], y2, sin_k[st][:], ALU.mult)
            nc.vector.tensor_tensor(kr_t[:, 0:32], t1[:], t2[:], ALU.subtract)
            nc.vector.tensor_tensor(t1[:], y2, cos_k[st][:], ALU.mult)
            nc.vector.tensor_tensor(t2[:], y1, sin_k[st][:], ALU.mult)
            nc.vector.tensor_tensor(kr_t[:, 32:64], t1[:], t2[:], ALU.add)
            for rt in range(KR // 128):
                tp = tr_ps.tile([128, 128], F, tag="tr", name=f"kvtr{st}_{rt}")
                nc.tensor.transpose(tp[:], ckvn[:, ts(rt, 128)], ident[:])
                sb_t = trsb.tile([128, 128], BF, tag="trsb", name=f"kvtrs{st}_{rt}")
                nc.any.tensor_copy(sb_t[:], tp[:])
                nc.gpsimd.dma_start(g1kv_src.ap()[ts(rt, 128), ts(st, 128)], sb_t[:])
            tpk = trk_ps.tile([64, 128], F, tag="trk", name=f"kvtrk{st}")
            nc.tensor.transpose(tpk[:], kr_t[:], ident[:])
            sb_k = trsb.tile([64, 128], BF, tag="trsbk", name=f"kvtrks{st}")
            nc.any.tensor_copy(sb_k[:], tpk[:])
            nc.gpsimd.dma_start(g1kv_src.ap()[KR : KR + DR, ts(st, 128)], sb_k[:])
        ctx0a.close()

        nc.gpsimd.collective_compute(
            "AllGather", ALU.bypass,
            ins=[g1kv_src.ap().opt()], outs=[g1kv.ap().opt()], replica_groups=RG,
        )

        # --- q_a (chunks of 512 cols) ---
        ctx0b = ExitStack()
        qa_ps = ctx0b.enter_context(tc.tile_pool(name="qa_ps", bufs=6, space="PSUM"))
        tr2_ps = ctx0b.enter_context(tc.tile_pool(name="tr2_ps", bufs=2, space="PSUM"))
        tr2sb = ctx0b.enter_context(tc.tile_pool(name="tr2sb", bufs=3))
        qch = [[None] * 3 for _ in range(2)]
        for ch in range(3):
            for st in range(2):
                qch[st][ch] = qa_ps.tile(
                    [128, 512], F, tag="qa_ps", name=f"qaps{st}_{ch}"
                )
        for ch in range(3):
            for ktg in range(NKT // 8):
                w = wp0.tile([128, 8, 512], BF, tag="qaw", name=f"qaw{ch}_{ktg}")
                nc.scalar.dma_start(w[:], qawT.ap()[ch, :, ds(8 * ktg, 8), :])
                for kk in range(8):
                    kt = ktg * 8 + kk
                    for st in range(2):
                        nc.tensor.matmul(
                            qch[st][ch][:], xt[:, kt, ts(st, 128)], w[:, kk],
                            start=(kt == 0), stop=(kt == NKT - 1),
                        )
        for st in range(2):
            accs = []
            scr = p0.tile([128, 512], F, tag="qascr", name=f"qascr{st}")
            for ch in range(3):
                a = p0.tile([128, 1], F, tag="qaacc", name=f"qaacc{st}_{ch}")
                nc.scalar.activation(scr[:], qch[st][ch][:], AF.Square, accum_out=a[:])
                accs.append(a)
            nc.vector.tensor_tensor(accs[0][:], accs[0][:], accs[1][:], ALU.add)
            nc.vector.tensor_tensor(accs[0][:], accs[0][:], accs[2][:], ALU.add)
            stdv = p0.tile([128, 1], F, tag="qastd", name=f"qastd{st}")
            nc.scalar.activation(stdv[:], accs[0][:], AF.Sqrt, bias=eps_t[:], scale=1.0 / QR)
            rinv = p0.tile([128, 1], F, tag="qarinv", name=f"qarinv{st}")
            nc.vector.reciprocal(rinv[:], stdv[:])
            qn = p0.tile([128, QR], F, tag="qn", name=f"qn{st}")
            for ch in range(3):
                nc.vector.tensor_scalar_mul(qn[:, ts(ch, 512)], qch[st][ch][:], rinv[:])
            for rt in range(NR):
                tp = tr2_ps.tile([128, 128], F, tag="tr2", name=f"qtr{st}_{rt}")
                nc.tensor.transpose(tp[:], qn[:, ts(rt, 128)], ident[:])
                sb_t = tr2sb.tile([128, 128], BF, tag="tr2sb", name=f"qtrs{st}_{rt}")
                nc.any.tensor_copy(sb_t[:], tp[:])
                nc.gpsimd.dma_start(g1q_src.ap()[ts(rt, 128), ts(st, 128)], sb_t[:])
        ctx0b.close()
        ctx0.close()

        nc.gpsimd.collective_compute(
            "AllGather", ALU.bypass,
            ins=[g1q_src.ap().opt()], outs=[g1q.ap().opt()], replica_groups=RG,
        )

        # ---------------- shared latents on-chip ----------------
        ctx_att = ExitStack()
        attb = ctx_att.enter_context(tc.tile_pool(name="attb", bufs=1))

        kpe_rep = attb.tile([128, S], BF, name="kpe_rep")
        for half in range(2):
            nc.sync.dma_start(
                kpe_rep[ds(64 * half, 64), :].rearrange("p (c s) -> p c s", c=NCORES),
                g1kv.ap()[:, KR : KR + DR, :].rearrange("c p s -> p c s"),
            )

        qnope = [attb.tile([128, S], BF, name=f"qnope{h}") for h in range(HPC)]
        qfpe = [attb.tile([128, S], BF, name=f"qfpe{p}") for p in range(2)]
        v_sb = attb.tile([128, S // 128, 512], BF, name="v_sb")
        kn_all = [attb.tile([128, S], BF, name=f"kn{h}") for h in range(HPC)]

        # ---------------- phase 1: kv decompression (hides q AllGather) --
        ctxd = ExitStack()
        ckvp = ctxd.enter_context(tc.tile_pool(name="ckvp", bufs=1))
        dec_ps = ctxd.enter_context(tc.tile_pool(name="dec_ps", bufs=2, space="PSUM"))
        ckv_t = []
        for r in range(NKR):
            t = ckvp.tile([128, S], BF, name=f"ckv{r}")
            nc.sync.dma_start(
                t[:].rearrange("p (c s) -> p c s", c=NCORES),
                g1kv.ap()[:, ts(r, 128), :].rearrange("c p s -> p c s"),
            )
            ckv_t.append(t)
        kvbv, kvbn = [], []
        for r in range(NKR):
            tv = ckvp.tile([128, 512], BF, name=f"kvbv{r}")
            nc.sync.dma_start(tv[:], kvbwT.ap()[ts(r, 128), 512:1024])
            kvbv.append(tv)
            tn = ckvp.tile([128, 512], BF, name=f"kvbn{r}")
            nc.sync.dma_start(tn[:], kvbwT.ap()[ts(r, 128), 0:512])
            kvbn.append(tn)

        for st in range(S // 128):
            ps = dec_ps.tile([128, 512], F, tag="dec", name=f"vps{st}")
            for r in range(NKR):
                nc.tensor.matmul(
                    ps[:], ckv_t[r][:, ts(st, 128)], kvbv[r][:],
                    start=(r == 0), stop=(r == NKR - 1),
                )
            nc.any.tensor_copy(v_sb[:, st, :], ps[:])
        for h in range(HPC):
            for sb in range(4):
                ps = dec_ps.tile([128, 512], F, tag="dec", name=f"knps{h}_{sb}")
                for r in range(NKR):
                    nc.tensor.matmul(
                        ps[:], kvbn[r][:, ts(h, 128)], ckv_t[r][:, ts(sb, 512)],
                        start=(r == 0), stop=(r == NKR - 1),
                    )
                nc.any.tensor_copy(kn_all[h][:, ts(sb, 512)], ps[:])
        ctxd.close()

        # ---------------- phase 2: q_b projection (+ q rope) ----------------
        ctx2 = ExitStack()
        qrp = ctx2.enter_context(tc.tile_pool(name="qrp", bufs=13))
        qbwp = ctx2.enter_context(tc.tile_pool(name="qbwp", bufs=1))
        ropep = ctx2.enter_context(tc.tile_pool(name="ropep", bufs=2))
        qb_ps = ctx2.enter_context(tc.tile_pool(name="qb_ps", bufs=3, space="PSUM"))

        # q_b weights resident in SBUF, loaded once (3 MB)
        qbw = []
        for r in range(NR):
            w = qbwp.tile([128, 768], BF, name=f"qbw{r}")
            nc.scalar.dma_start(w[:], qbwT.ap()[ts(r, 128), :])
            qbw.append(w)

        JORD = [0, 1, 2, 3, 4, 5]
        for sb in range(4):
            qr_tiles = []
            for r in range(NR):
                t = qrp.tile([128, 512], BF, tag="qr", name=f"qr{sb}_{r}")
                nc.sync.dma_start(
                    t[:].rearrange("p (c s) -> p c s", c=2),
                    g1q.ap()[2 * sb : 2 * sb + 2, ts(r, 128), :]
                    .rearrange("c p s -> p c s"),
                )
                qr_tiles.append(t)
            ps_of = {}
            for j in JORD:
                psj = qb_ps.tile([128, 512], F, tag="qb_ps", name=f"qbps{sb}_{j}")
                ps_of[j] = psj
                for r in range(NR):
                    nc.tensor.matmul(
                        psj[:], qbw[r][:, ts(j, 128)], qr_tiles[r][:],
                        start=(r == 0), stop=(r == NR - 1),
                    )
                if j < 4:
                    nc.any.tensor_copy(qnope[j][:, ts(sb, 512)], psj[:])
                else:
                    p = j - 4
                    t1 = ropep.tile([128, 512], F, tag="rope1", name=f"rp1_{sb}_{p}")
                    t2 = ropep.tile([128, 512], F, tag="rope2", name=f"rp2_{sb}_{p}")
                    nc.vector.tensor_tensor(
                        t1[:], psj[:], cos_q[:, ts(sb, 512)], ALU.mult
                    )
                    for o in (0, 64):
                        nc.vector.tensor_tensor(
                            t2[ds(o, 32), :], psj[ds(o + 32, 32), :],
                            ssin_q[ds(o, 32), ts(sb, 512)], ALU.mult,
                        )
                        nc.vector.tensor_tensor(
                            t2[ds(o + 32, 32), :], psj[ds(o, 32), :],
                            ssin_q[ds(o + 32, 32), ts(sb, 512)], ALU.mult,
                        )
                    nc.vector.tensor_tensor(
                        qfpe[p][:, ts(sb, 512)], t1[:], t2[:], ALU.add
                    )
        ctx2.close()

        # ---------------- phase 3: attention + per-head AllToAll ---------
        ctx3 = ExitStack()
        probp = ctx3.enter_context(tc.tile_pool(name="probp", bufs=6))
        invp = ctx3.enter_context(tc.tile_pool(name="invp", bufs=2))
        psump = ctx3.enter_context(tc.tile_pool(name="psump", bufs=4))
        attp = ctx3.enter_context(tc.tile_pool(name="attp", bufs=2))
        denp = ctx3.enter_context(tc.tile_pool(name="denp", bufs=2))
        ctx3p = ExitStack()
        sc_ps = ctx3p.enter_context(tc.tile_pool(name="sc_ps", bufs=3, space="PSUM"))
        av_ps = ctx3p.enter_context(tc.tile_pool(name="av_ps", bufs=4, space="PSUM"))
        tail_ps = ctx3p.enter_context(tc.tile_pool(name="tail_ps", bufs=1, space="PSUM"))

        # The four q-blocks of a head run phase-staggered so the PE always
        # has 2-4 independent score->exp->AV chains in flight (keeps the
        # activity clock-gate warm).
        QOFF = {3: 0, 2: 2, 1: 4, 0: 6}
        for h in range(HPC):
            kn = kn_all[h]
            pe_rhs = qfpe[h // 2][ds(64 * (h % 2), 64), :]
            pe_lhs = kpe_rep[ds(64 * (h % 2), 64), :]
            avt_of, psum_of = {}, {}
            for s in range(16):
                for qb in (3, 2, 1, 0):
                    kt = s - QOFF[qb]
                    nkt = 4 * (qb + 1)
                    if not (0 <= kt < nkt):
                        continue
                    if kt == 0:
                        avt_of[qb] = av_ps.tile(
                            [128, 512], F, tag="av", name=f"av{h}_{qb}"
                        )
                        psum_of[qb] = psump.tile(
                            [128, 512], R, tag="psum", name=f"psum{h}_{qb}"
                        )
                    avt, psum = avt_of[qb], psum_of[qb]
                    trim = max(0, 128 * (kt - 4 * qb))
                    qsl = ds(512 * qb + trim, 512 - trim)
                    sct = sc_ps.tile([128, 512], F, tag="sc", name=f"sc{h}{qb}_{kt}")
                    nc.tensor.matmul(
                        sct[:, trim:512], kn[:, ts(kt, 128)], qnope[h][:, qsl],
                        start=True, stop=False,
                    )
                    nc.tensor.matmul(
                        sct[:, trim:512], pe_lhs[:, ts(kt, 128)], pe_rhs[:, qsl],
                        start=False, stop=True,
                    )
                    prob = probp.tile([128, 512], BF, tag="prob", name=f"pr{h}{qb}_{kt}")
                    nc.scalar.activation(prob[:, trim:512], sct[:, trim:512], AF.Exp)
                    if kt >= 4 * qb:
                        nc.vector.tensor_tensor(
                            prob[:, trim : trim + 128],
                            prob[:, trim : trim + 128],
                            triu[:],
                            ALU.mult,
                        )
                    nc.tensor.matmul(
                        avt[:, trim:512], v_sb[:, kt, ts(h, 128)], prob[:, trim:512],
                        start=(kt == 0), stop=(kt == nkt - 1),
                    )
                    if kt == 0:
                        nc.vector.tensor_copy(psum[:], prob[:])
                    else:
                        nc.vector.tensor_tensor(
                            psum[:, trim:512], psum[:, trim:512],
                            prob[:, trim:512], ALU.add,
                        )
                    if kt == nkt - 1:
                        # att = avt * exp(-ln(den)): Ln/Exp share one ACT
                        # table set (no DVE reciprocal at 8 cyc/elem, no
                        # table thrash); broadcast on the idle gpsimd.
                        dent = tail_ps.tile([1, 512], F, tag="tail", name=f"den{h}_{qb}")
                        nc.tensor.matmul(
                            dent[:], ones_col[:], psum[:], start=True, stop=True
                        )
                        lden = denp.tile([1, 512], F, tag="lden", name=f"lden{h}_{qb}")
                        nc.scalar.activation(lden[:], dent[:], AF.Ln)
                        ir = denp.tile([1, 512], F, tag="invr", name=f"invr{h}_{qb}")
                        nc.scalar.activation(ir[:], lden[:], AF.Exp, scale=-1.0)
                        einv = invp.tile([128, 512], F, tag="einv", name=f"einv{h}_{qb}")
                        nc.gpsimd.partition_broadcast(einv[:], ir[:])
                        att = attp.tile([128, 512], BF, tag="att", name=f"att{h}_{qb}")
                        nc.vector.tensor_tensor(att[:], avt[:], einv[:], ALU.mult)
                        nc.gpsimd.dma_start(a2a_src[h].ap()[qb], att[:])
                        nc.gpsimd.dma_start(a2a_src[h].ap()[qb + 4], att[:])

            nc.gpsimd.collective_compute(
                "AllToAll", ALU.bypass,
                ins=[a2a_src[h].ap().opt()], outs=[a2a_out[h].ap().opt()],
                replica_groups=RG,
            )

        ctx3p.close()

        # ---------------- phase 4: o_proj (my H-half x my q-block) -------
        ctx5 = ExitStack()
        owp = ctx5.enter_context(tc.tile_pool(name="owp", bufs=3))
        a2ap = ctx5.enter_context(tc.tile_pool(name="a2ap", bufs=1))
        osb = ctx5.enter_context(tc.tile_pool(name="osb", bufs=2))
        o_ps = ctx5.enter_context(tc.tile_pool(name="o_ps", bufs=2, space="PSUM"))

        # attention outputs for my q-block: nd chunk (h*8 + i) = head 4i+h.
        # Head 3's loads are emitted after the G1 passes so they don't
        # head-of-line block the o_proj weight stream behind AllToAll #3.
        a2a_sb = []
        for h in range(HPC - 1):
            for i in range(NCORES):
                t = a2ap.tile([128, QBLK], BF, name=f"a2a_{h}_{i}")
                nc.sync.dma_start(t[:], a2a_out[h].ap()[i])
                a2a_sb.append(t)

        # G1 (heads from A2A 0-2) runs while the last AllToAll is in flight;
        # G2 (A2A 3's 8 chunks) accumulates into DRAM afterwards.
        for ht in range(16):
            ow_t = owp.tile([128, 24, 128], BF, tag="ow", name=f"ow{ht}")
            nc.sync.dma_start(
                ow_t[:], owT.ap()[ht, 0:3072, :].rearrange("(nd p) j -> p nd j", p=128)
            )
            ps = o_ps.tile([128, QBLK], F, tag="o", name=f"ops{ht}")
            for nd in range(24):
                nc.tensor.matmul(
                    ps[:], ow_t[:, nd, :], a2a_sb[nd][:],
                    start=(nd == 0), stop=(nd == 23),
                )
            ost = osb.tile([128, QBLK], F, tag="osb", name=f"osb{ht}")
            nc.any.tensor_copy(ost[:], ps[:])
            nc.gpsimd.dma_start(out_d.ap()[ts(ht, 128), :], ost[:])
        for h in (HPC - 1,):
            for i in range(NCORES):
                t = a2ap.tile([128, QBLK], BF, name=f"a2a_{h}_{i}")
                nc.sync.dma_start(t[:], a2a_out[h].ap()[i])
                a2a_sb.append(t)
        for ht in range(16):
            ow_t = owp.tile([128, 8, 128], BF, tag="ow2", name=f"ow2_{ht}")
            nc.sync.dma_start(
                ow_t[:],
                owT.ap()[ht, 3072:4096, :].rearrange("(nd p) j -> p nd j", p=128),
            )
            ps = o_ps.tile([128, QBLK], F, tag="o", name=f"ops2_{ht}")
            for nd in range(8):
                nc.tensor.matmul(
                    ps[:], ow_t[:, nd, :], a2a_sb[24 + nd][:],
                    start=(nd == 0), stop=(nd == 7),
                )
            ost = osb.tile([128, QBLK], F, tag="osb", name=f"osb2_{ht}")
            nc.any.tensor_copy(ost[:], ps[:])
            nc.gpsimd.dma_start(
                out_d.ap()[ts(ht, 128), :], ost[:], accum_op=ALU.add
            )
        ctx5.close()
        ctx3.close()
        ctx_att.close()

    nc.compile()
    return nc


def _get_nc():
    global _CACHED_NC
    if _CACHED_NC is None:
        _CACHED_NC = _build_program()
    return _CACHED_NC


def _prep_inputs(hidden_states, position_ids, q_a_w, q_a_ln_w, q_b_w, kv_a_w,
                 kv_a_ln_w, kv_b_w, o_w):
    hidden_states = np.asarray(hidden_states, dtype=np.float32)
    position_ids = np.asarray(position_ids, dtype=np.int32)
    q_a_w = np.asarray(q_a_w, dtype=np.float32)
    q_a_ln_w = np.asarray(q_a_ln_w, dtype=np.float32)
    q_b_w = np.asarray(q_b_w, dtype=np.float32)
    kv_a_w = np.asarray(kv_a_w, dtype=np.float32)
    kv_a_ln_w = np.asarray(kv_a_ln_w, dtype=np.float32)
    kv_b_w = np.asarray(kv_b_w, dtype=np.float32)
    o_w = np.asarray(o_w, dtype=np.float32)

    bf = ml_dtypes.bfloat16
    xT_full = np.ascontiguousarray(hidden_states[0].T)          # (H, S)
    qawT = np.ascontiguousarray(q_a_w.T)                         # (H, QR)
    kvawT = np.ascontiguousarray(kv_a_w.T)                       # (H, KR+DR)
    # pre-rearranged for contiguous SBUF-layout DMA: (128, H/128, cols)
    kvawT_c = np.ascontiguousarray(
        kvawT.reshape(H // 128, 128, KR + DR).transpose(1, 0, 2)
    )
    qawT_c = np.ascontiguousarray(
        qawT.reshape(H // 128, 128, QR).transpose(1, 0, 2)      # (128, 32, QR)
        .reshape(128, H // 128, 3, 512).transpose(2, 0, 1, 3)   # (3, 128, 32, 512)
    )
    ident = np.eye(128, dtype=np.float32)
    triu = np.triu(np.ones((128, 128), dtype=np.float32)).astype(bf)
    inv_freq = (1.0 / (THETA ** (np.arange(0, DR, 2) / DR))).astype(np.float32)
    if32 = np.tile(inv_freq[None, :], (128, 1)).astype(np.float32)
    if128 = np.tile(inv_freq, 4)[:, None].astype(np.float32)
    owT_full = np.ascontiguousarray(o_w.T)                       # (N*DV, H)

    in_maps = []
    for c in range(NCORES):
        heads = slice(HPC * c, HPC * (c + 1))
        qb = q_b_w.reshape(N_HEADS, QD, QR)[heads]               # (4, 192, QR)
        nope = qb[:, :DN, :].reshape(HPC * DN, QR)
        pe = qb[:, DN:, :]
        pe_d = np.concatenate([pe[:, 0::2, :], pe[:, 1::2, :]], axis=1)  # (4,64,QR)
        cols = np.concatenate(
            [nope, pe_d.reshape(HPC * DR, QR)], axis=0
        )                                                        # (768, QR)
        qbwT_c = np.ascontiguousarray((cols * (SCALE * q_a_ln_w[None, :])).T)

        kvb = kv_b_w.reshape(N_HEADS, DN + DV, KR)[heads]
        kcols = np.concatenate(
            [kvb[:, :DN, :].reshape(HPC * DN, KR),
             kvb[:, DN:, :].reshape(HPC * DV, KR)],
            axis=0,
        )                                                        # (1024, KR)
        kvbwT_c = np.ascontiguousarray((kcols * kv_a_ln_w[None, :]).T)

        # o_proj weights: rows ordered (h_loc, src_rank) -> head 4*i + h_loc,
        # column slice = this core's H-half, chunked per 128-col ht pass.
        hhalf = c // 4
        row_order = np.empty((HPC * NCORES,), dtype=np.int64)
        for h_loc in range(HPC):
            for i in range(NCORES):
                row_order[h_loc * NCORES + i] = 4 * i + h_loc
        owT_rows = owT_full.reshape(N_HEADS, DV, H)[row_order].reshape(
            N_HEADS * DV, H
        )[:, HHALF * hhalf : HHALF * (hhalf + 1)]                # (4096, 2048)
        owT_c = np.ascontiguousarray(
            owT_rows.reshape(N_HEADS * DV, 16, 128).transpose(1, 0, 2)
        )                                                        # (16, 4096, 128)

        xT_c = np.ascontiguousarray(
            xT_full[:, SL * c : SL * (c + 1)].reshape(H // 128, 128, SL)
            .transpose(1, 0, 2)
        )                                                        # (128, 32, SL)
        in_maps.append(
            {
                "xT": xT_c.astype(bf),
                "qawT": qawT_c.astype(bf),
                "kvawT": kvawT_c.astype(bf),
                "qbwT": qbwT_c.astype(bf),
                "kvbwT": kvbwT_c.astype(bf),
                "owT": owT_c.astype(bf),
                "pos_all": np.ascontiguousarray(position_ids.reshape(1, S)),
                "pos_loc": np.ascontiguousarray(
                    position_ids.reshape(-1)[SL * c : SL * (c + 1)]
                ),
                "ident": ident,
                "triu": triu,
                "if32": if32,
                "if128": if128,
            }
        )
    return in_maps


def kernel(**inputs):
    global LAST_RESULT
    nc = _get_nc()
    in_maps = _prep_inputs(**inputs)
    res = run_bass_kernel_spmd(nc, in_maps, list(range(NCORES)))
    LAST_RESULT = res
    outT = np.empty((H, S), dtype=np.float32)
    for c in range(NCORES):
        hhalf, qblk = c // 4, c % 4
        outT[HHALF * hhalf : HHALF * (hhalf + 1),
             QBLK * qblk : QBLK * (qblk + 1)] = res.results[c]["out"]
    return outT.T[None].astype(np.float32)
